# revision 1
# baseline (speedup 1.0000x reference)
"""Trainium2 Bass kernel for nn_EntropyMaskGate.

Pipeline per core (2 images, batch-sharded over 8 cores):
  conv1 (1x1, 256->64) -> gelu -> grouped 3x3 conv (SAME, 8 groups) -> gelu
  -> conv3 (1x1, 64->256) + bias  = entropy_scores            [output 2]
  block 2x2 sums of scores -> per-(b,c)-row 256th-smallest threshold
  (value bisection + exact top-8 finisher) -> binary keep mask, upsampled 2x2
                                                              [output 1]

Forward-pass note: the reference's STE expression (sg(hard) - sg(soft) + soft)
is exactly `hard` in fp32 round-to-nearest (soft is clipped to [0,1]), so the
mask output is the upsampled hard mask; the soft path is numerically dead.
"""

import numpy as np

import concourse.bass as bass
import concourse.mybir as mybir
from concourse import bacc, bass_utils
from concourse.tile import TileContext

F32 = mybir.dt.float32
I32 = mybir.dt.int32
U8 = mybir.dt.uint8
AF = mybir.ActivationFunctionType
OP = mybir.AluOpType

B, C, H, W = 16, 256, 64, 64
MID, GROUPS = 64, 8
N_CORES = 8
IMGS = B // N_CORES          # 2 images per core
HW = H * W                   # 4096
NBLK = 1024                  # 32*32 blocks per (b,c) row
KEEP = 256                   # blocks kept per row
T_BISECT = 13                # bisection iterations
BIGNEG = -(2.0 ** 96)        # power-of-two penalty (exact count recovery)

# matmul input dtype: float32r streams at bf16 rate (>=256 cols) but is only
# used if it turns out numerically fp32-exact-ish; float32 is the 4x fallback.
MM_DT = F32
TRACE = False
LAST_RESULTS = None
import os as _os
DBG_TAPS = int(_os.environ.get("KM_TAPS", "9"))
DBG_TBI = int(_os.environ.get("KM_TBI", str(T_BISECT)))
DBG_SKIPFIN = int(_os.environ.get("KM_SKIPFIN", "0"))
DBG_SKIPSEARCH = int(_os.environ.get("KM_SKIPSEARCH", "0"))


def _mm(ap):
    return ap.bitcast(MM_DT) if MM_DT is not F32 else ap


def build_nc(repeat=1):
    nc = bacc.Bacc("TRN2", target_bir_lowering=False, debug=False,
                   num_devices=N_CORES)

    feats_d = nc.dram_tensor("features", [IMGS, C, HW], F32, kind="ExternalInput").ap()
    w1t_d = nc.dram_tensor("w1t", [128, 2, 128], F32, kind="ExternalInput").ap()
    w2p_d = nc.dram_tensor("w2p", [128, 3, MID], F32, kind="ExternalInput").ap()
    w2s_d = nc.dram_tensor("w2s", [MID, 3, MID], F32, kind="ExternalInput").ap()
    w3t_d = nc.dram_tensor("w3t", [MID, C], F32, kind="ExternalInput").ap()
    b1_d = nc.dram_tensor("b1", [128, 1], F32, kind="ExternalInput").ap()
    b2_d = nc.dram_tensor("b2", [MID, 1], F32, kind="ExternalInput").ap()
    b3s_d = nc.dram_tensor("b3s", [128, 2], F32, kind="ExternalInput").ap()
    b3x4_d = nc.dram_tensor("b3x4", [128, 2], F32, kind="ExternalInput").ap()
    scores_d = nc.dram_tensor("scores", [IMGS, C, HW], F32, kind="ExternalOutput").ap()
    mask_d = nc.dram_tensor("mask", [IMGS, C, HW], F32, kind="ExternalOutput").ap()

    with TileContext(nc) as tc:
        for _rep in range(repeat):
            _build(nc, tc, feats_d, w1t_d, w2p_d, w2s_d, w3t_d, b1_d, b2_d,
                   b3s_d, b3x4_d, scores_d, mask_d)
    nc.compile()
    return nc


def _build(nc, tc, feats_d, w1t_d, w2p_d, w2s_d, w3t_d, b1_d, b2_d, b3s_d,
           b3x4_d, scores_d, mask_d):
    cpool = tc.alloc_tile_pool(name="consts", bufs=1)
    xpool = tc.alloc_tile_pool(name="x", bufs=2)
    h1pool = tc.alloc_tile_pool(name="h1", bufs=2)
    h2pool = tc.alloc_tile_pool(name="h2", bufs=6)
    spool = tc.alloc_tile_pool(name="s", bufs=3)
    ipool = tc.alloc_tile_pool(name="impp", bufs=2)
    scrpool = tc.alloc_tile_pool(name="scr", bufs=2)
    hpool = tc.alloc_tile_pool(name="hard", bufs=2)
    stpool = tc.alloc_tile_pool(name="stats", bufs=1)
    ps1 = tc.alloc_tile_pool(name="ps1", bufs=2, space="PSUM")
    ps2 = tc.alloc_tile_pool(name="ps2", bufs=2, space="PSUM")
    ps3 = tc.alloc_tile_pool(name="ps3", bufs=2, space="PSUM")
    psq = tc.alloc_tile_pool(name="psq", bufs=2, space="PSUM")

    # ---- constants ----
    w1_sb = cpool.tile([128, 2, 128], F32, name="w1", tag="w1")
    nc.sync.dma_start(out=w1_sb[:], in_=w1t_d[:])
    w2p_sb = cpool.tile([128, 3, MID], F32, name="w2p", tag="w2p")
    nc.sync.dma_start(out=w2p_sb[:], in_=w2p_d[:])
    w2s_sb = cpool.tile([MID, 3, MID], F32, name="w2s", tag="w2s")
    nc.sync.dma_start(out=w2s_sb[:], in_=w2s_d[:])
    w3_sb = cpool.tile([MID, C], F32, name="w3", tag="w3")
    nc.sync.dma_start(out=w3_sb[:], in_=w3t_d[:])
    b1_sb = cpool.tile([128, 1], F32, name="b1", tag="b1")
    nc.sync.dma_start(out=b1_sb[:], in_=b1_d[:])
    b2_sb = cpool.tile([MID, 1], F32, name="b2", tag="b2")
    nc.sync.dma_start(out=b2_sb[:], in_=b2_d[:])
    b3s_sb = cpool.tile([128, 2], F32, name="b3s", tag="b3s")
    nc.sync.dma_start(out=b3s_sb[:], in_=b3s_d[:])
    b3x4_sb = cpool.tile([128, 2], F32, name="b3x4", tag="b3x4")
    nc.sync.dma_start(out=b3x4_sb[:], in_=b3x4_d[:])

    iota_i = cpool.tile([128, 8], I32, name="iotai", tag="iotai")
    nc.gpsimd.iota(iota_i[:], pattern=[[1, 8]], base=0, channel_multiplier=0)
    iotaneg = cpool.tile([128, 8], F32, name="iotan", tag="iotan")
    nc.vector.tensor_copy(iotaneg[:], iota_i[:])
    nc.vector.tensor_scalar(iotaneg[:], iotaneg[:], -1.0, None, op0=OP.mult)
    negbig = cpool.tile([128, NBLK], F32, name="negbig", tag="negbig")
    nc.vector.memset(negbig[:], BIGNEG)

    # ---- per-image threshold search + mask (runs as soon as that image's
    # impp is ready, overlapping the next image's conv stack on PE/ACT).
    # All counting on DVE (exact is_le) so ACT keeps its Gelu table hot. ----
    def _threshold_and_mask(img, impp):
        def st_tile(tag, cols=2):
            return stpool.tile([128, cols], F32, name=f"{tag}{img}",
                               tag=f"{tag}{img}")
        sm, sq, mu, msq, var = (st_tile(t) for t in
                                ["sm", "sq", "mu", "msq", "var"])
        sig, wid, lo, hi, mid = (st_tile(t) for t in
                                 ["sig", "wid", "lo", "hi", "mid"])
        cnt, cntlos, jneg, thrneg, thr = (st_tile(t) for t in
                                          ["cnt", "cntlos", "jneg", "thrneg",
                                           "thr"])
        sgek = stpool.tile([128, 2], U8, name=f"sgek{img}", tag=f"sgek{img}")
        sltk = stpool.tile([128, 2], U8, name=f"sltk{img}", tag=f"sltk{img}")
        imps = [impp[:, ct, :] for ct in range(2)]

        # row stats -> Chebyshev bracket [mu - 2.1s, mu + 2.1s]
        for rt, imp in enumerate(imps):
            scr = scrpool.tile([128, NBLK], F32, name="scr", tag="scr")
            nc.scalar.activation(scr[:], imp, AF.Copy,
                                 accum_out=sm[:, rt:rt + 1])
            scr2 = scrpool.tile([128, NBLK], F32, name="scrA", tag="scrA")
            nc.scalar.activation(scr2[:], imp, AF.Square,
                                 accum_out=sq[:, rt:rt + 1])
        nc.vector.tensor_scalar(mu[:], sm[:], 1.0 / NBLK, None, op0=OP.mult)
        nc.vector.tensor_scalar(msq[:], sq[:], 1.0 / NBLK, None, op0=OP.mult)
        nc.vector.tensor_mul(var[:], mu[:], mu[:])
        nc.vector.tensor_sub(var[:], msq[:], var[:])
        nc.vector.tensor_scalar(var[:], var[:], 0.0, None, op0=OP.max)
        nc.scalar.activation(sig[:], var[:], AF.Sqrt)
        nc.vector.tensor_scalar(wid[:], sig[:], 2.1, 1e-18, op0=OP.mult,
                                op1=OP.add)
        nc.vector.tensor_sub(lo[:], mu[:], wid[:])
        nc.vector.tensor_add(hi[:], mu[:], wid[:])
        nc.vector.tensor_add(mid[:], lo[:], hi[:])
        nc.vector.tensor_scalar(mid[:], mid[:], 0.5, None, op0=OP.mult)

        for it in range(0 if DBG_SKIPSEARCH else DBG_TBI):
            for rt in range(2):
                scr = scrpool.tile([128, NBLK], F32, name="scr", tag="scr")
                nc.vector.tensor_scalar(scr[:], imps[rt], mid[:, rt:rt + 1],
                                        None, op0=OP.is_le, op1=OP.add,
                                        accum_out=cnt[:, rt:rt + 1])
            nc.vector.tensor_scalar(sgek[:], cnt[:], float(KEEP), None,
                                    op0=OP.is_ge)
            nc.vector.copy_predicated(hi[:], sgek[:], mid[:])
            nc.vector.tensor_scalar(sltk[:], cnt[:], float(KEEP), None,
                                    op0=OP.is_lt)
            nc.vector.copy_predicated(lo[:], sltk[:], mid[:])
            nc.vector.tensor_add(mid[:], lo[:], hi[:])
            nc.vector.tensor_scalar(mid[:], mid[:], 0.5, None, op0=OP.mult)

        # exact finisher: v_k = (256 - cnt_le(lo))-th smallest in (lo, hi]
        for rt, imp in enumerate(imps if not DBG_SKIPFIN else []):
            cand = scrpool.tile([128, NBLK], F32, name="scr", tag="scr")
            nc.vector.tensor_scalar(cand[:], imp, -1.0, None, op0=OP.mult)
            mA = scrpool.tile([128, NBLK], U8, name="mA", tag="mA")
            nc.vector.tensor_scalar(mA[:], imp, hi[:, rt:rt + 1], None,
                                    op0=OP.is_gt)
            nc.vector.copy_predicated(cand[:], mA[:], negbig[:])
            mB = scrpool.tile([128, NBLK], U8, name="mB", tag="mB")
            nc.vector.tensor_scalar(mB[:], imp, lo[:, rt:rt + 1], None,
                                    op0=OP.is_le, op1=OP.add,
                                    accum_out=cntlos[:, rt:rt + 1])
            nc.vector.copy_predicated(cand[:], mB[:], negbig[:])
            top8 = stpool.tile([128, 8], F32, name=f"top8_{img}{rt}",
                               tag=f"top8_{img}{rt}")
            nc.vector.max(out=top8[:], in_=cand[:])
            nc.vector.tensor_scalar(jneg[:, rt:rt + 1], cntlos[:, rt:rt + 1],
                                    -255.0, None, op0=OP.add)
            eq8 = stpool.tile([128, 8], F32, name=f"eq8_{img}{rt}",
                              tag=f"eq8_{img}{rt}")
            nc.vector.tensor_scalar(eq8[:], iotaneg[:], jneg[:, rt:rt + 1],
                                    None, op0=OP.is_equal)
            scr8 = stpool.tile([128, 8], F32, name=f"scr8_{img}{rt}",
                               tag=f"scr8_{img}{rt}")
            nc.vector.tensor_mul(scr8[:], top8[:], eq8[:])
            nc.vector.tensor_reduce(thrneg[:, rt:rt + 1], scr8[:],
                                    axis=mybir.AxisListType.X, op=OP.add)
        if DBG_SKIPFIN:
            nc.vector.memset(thr[:], 0.0)
        else:
            nc.vector.tensor_scalar(thr[:], thrneg[:], -1.0, None,
                                    op0=OP.mult)

        # hard mask, upsampled 2x in W on-chip, 2x in H via dual DMA
        for rt, imp in enumerate(imps):
            ct = rt
            hard = hpool.tile([128, 2048], F32, name="hard", tag="hard")
            impb = imp.rearrange("p (i j) -> p i j", j=32).unsqueeze(3) \
                      .broadcast_to([128, 32, 32, 2])
            nc.vector.tensor_scalar(
                hard[:].rearrange("p (i j r) -> p i j r", j=32, r=2),
                impb, thr[:, rt:rt + 1], None, op0=OP.is_le)
            mview = mask_d[img].rearrange("(t c) (i r w) -> c t i r w",
                                          t=2, r=2, w=64)
            hv = hard[:].rearrange("p (i w) -> p i w", w=128)
            nc.sync.dma_start(out=mview[:, ct, :, 0, :], in_=hv)
            nc.sync.dma_start(out=mview[:, ct, :, 1, :], in_=hv)

    # ---- per-image conv stack ----
    impp_tiles = []
    for img in range(IMGS):
        x0 = xpool.tile([128, HW], F32, name="x0", tag="x0")
        x1 = xpool.tile([128, HW], F32, name="x1", tag="x1")
        for dc in range(4):
            cs = dc * (HW // 4)
            nc.sync.dma_start(out=x0[:, cs:cs + HW // 4],
                              in_=feats_d[img, 0:128, cs:cs + HW // 4])
            nc.sync.dma_start(out=x1[:, cs:cs + HW // 4],
                              in_=feats_d[img, 128:256, cs:cs + HW // 4])

        # h1 with one-pixel zero halo, [66 x 66]. Partitions 0-63 hold h1;
        # partitions 64-127 hold a copy shifted one column left
        # (u[r, c] = h1pad[r, c+1]) so a single full-height AP feeds the
        # (dy,0)+(dy,1) tap pair of conv2 as one K=128 matmul.
        h1t = h1pool.tile([128, 66 * 66], F32, name="h1", tag="h1")
        h1v = h1t[0:64, :].rearrange("p (r c) -> p r c", c=66)
        h1u = h1t[64:128, :].rearrange("p (r c) -> p r c", c=66)
        nc.vector.memset(h1v[:, 0:1, :], 0.0)
        nc.vector.memset(h1v[:, 65:66, :], 0.0)
        nc.vector.memset(h1v[:, 1:65, 0:1], 0.0)
        nc.vector.memset(h1v[:, 1:65, 65:66], 0.0)
        nc.vector.memset(h1u[:, 0:1, :], 0.0)
        nc.vector.memset(h1u[:, 65:66, :], 0.0)
        nc.vector.memset(h1u[:, 1:65, 64:65], 0.0)

        # conv1 (1x1, M duplicated) + gelu into both h1 copies
        for ci in range(8):
            pt = ps1.tile([128, 512], F32, name="ps1", tag="ps1")
            cs = ci * 512
            nc.tensor.matmul(pt[:], _mm(w1_sb[:, 0, :]), _mm(x0[:, cs:cs + 512]),
                             start=True, stop=False)
            nc.tensor.matmul(pt[:], _mm(w1_sb[:, 1, :]), _mm(x1[:, cs:cs + 512]),
                             start=False, stop=True)
            nc.scalar.activation(h1v[:, 1 + 8 * ci:9 + 8 * ci, 1:65],
                                 pt[0:64, :].rearrange("p (r c) -> p r c", c=64),
                                 AF.Gelu, bias=b1_sb[0:64, 0:1])
            nc.scalar.activation(h1u[:, 1 + 8 * ci:9 + 8 * ci, 0:64],
                                 pt[64:128, :].rearrange("p (r c) -> p r c", c=64),
                                 AF.Gelu, bias=b1_sb[64:128, 0:1])

        # conv2 (grouped 3x3 SAME) + gelu -> h2 in 16-row tiles;
        # then conv3 + bias -> scores, and block-summed conv3 -> impp
        impp = ipool.tile([128, 2, NBLK], F32, name="impp", tag="impp")
        impp_tiles.append(impp)
        h2_tiles = []
        for c2 in range(4):
            h2t = h2pool.tile([MID, 1024], F32, name="h2", tag="h2")
            h2_tiles.append(h2t)
            for half in range(2):
                ci = 2 * c2 + half
                r0 = 8 * ci
                pt = ps2.tile([MID, 512], F32, name="ps2", tag="ps2")
                h1f = h1t[:].rearrange("p (r c) -> p r c", c=66)
                nmm = min(6, DBG_TAPS)
                for t in range(nmm):
                    if t < 3:      # pair (dy,0)+(dy,1), K=128 over both copies
                        dy = t
                        lhsT = w2p_sb[:, dy, :]
                        rhs = h1f[:, r0 + dy:r0 + dy + 8, 0:64]
                    else:          # single (dy,2), K=64 lower copy
                        dy = t - 3
                        lhsT = w2s_sb[:, dy, :]
                        rhs = h1v[:, r0 + dy:r0 + dy + 8, 2:66]
                    nc.tensor.matmul(
                        pt[:].rearrange("p (r c) -> p r c", c=64),
                        _mm(lhsT), _mm(rhs),
                        start=(t == 0), stop=(t == nmm - 1))
                nc.scalar.activation(h2t[:, half * 512:(half + 1) * 512], pt[:],
                                     AF.Gelu, bias=b2_sb[:, 0:1])
            # block-summed conv3 -> impp (2x2 sums via 4 accumulating matmuls)
            h2q = h2t[:].rearrange("p (i a j b) -> p i a j b", a=2, j=32, b=2)
            for mt in range(2):
                pq = psq.tile([128, 256], F32, name="psq", tag="psq")
                t = 0
                for a in range(2):
                    for b in range(2):
                        nc.tensor.matmul(pq[:].rearrange("p (i j) -> p i j", j=32),
                                         _mm(w3_sb[:, mt * 128:(mt + 1) * 128]),
                                         _mm(h2q[:, :, a, :, b]),
                                         start=(t == 0), stop=(t == 3))
                        t += 1
                nc.vector.tensor_scalar(impp[:, mt, c2 * 256:(c2 + 1) * 256],
                                        pq[:], b3x4_sb[:, mt:mt + 1], None,
                                        op0=OP.add)
        _threshold_and_mask(img, impp)
        # conv3: full-res scores, deferred so the PE work overlaps
        # this image's threshold search (which doesn't need them)
        for c2, h2t in enumerate(h2_tiles):
                for half in range(2):
                    ci = 2 * c2 + half
                    st = spool.tile([128, 2, 512], F32, name="s", tag="s")
                    for mt in range(2):
                        pt = ps3.tile([128, 512], F32, name="ps3", tag="ps3")
                        nc.tensor.matmul(pt[:], _mm(w3_sb[:, mt * 128:(mt + 1) * 128]),
                                                 _mm(h2t[:, half * 512:(half + 1) * 512]),
                                                 start=True, stop=True)
                        eng = nc.scalar if mt == 0 else nc.vector
                        if mt == 0:
                                nc.scalar.activation(st[:, mt, :], pt[:], AF.Identity,
                                                         bias=b3s_sb[:, mt:mt + 1])
                        else:
                                nc.vector.tensor_scalar(st[:, mt, :], pt[:],
                                                                b3s_sb[:, mt:mt + 1], None,
                                                                op0=OP.add)
                    nc.sync.dma_start(
                        out=scores_d[img].rearrange("(t c) w -> c t w", t=2)[
                                :, :, ci * 512:(ci + 1) * 512],
                        in_=st[:])


    # ---- (old batched search, replaced by per-image overlap) ----

    for _p in (psq, ps3, ps2, ps1, stpool, hpool, scrpool, ipool, spool,
               h2pool, h1pool, xpool, cpool):
        _p.release()


def _prep_weights(w1, b1, w2, b2, w3, b3):
    w1t = np.ascontiguousarray(
        w1[:, :, 0, 0].T.reshape(2, 128, MID).transpose(1, 0, 2)).astype(np.float32)
    w1d = np.concatenate([w1t, w1t], axis=2)      # [128, 2, 128]: M duplicated
    w2t = np.zeros((MID, 9, MID), np.float32)
    for m in range(MID):
        g = m // 8
        for dy in range(3):
            for dx in range(3):
                w2t[g * 8:(g + 1) * 8, 3 * dy + dx, m] = w2[m, :, dy, dx]
    # K=128 pairs: rows 0-63 tap (dy,0), rows 64-127 tap (dy,1); the upper
    # h1 copy is column-shifted -1 so one AP offset feeds both taps.
    w2p = np.stack([np.concatenate([w2t[:, 3 * dy + 0, :],
                                    w2t[:, 3 * dy + 1, :]], axis=0)
                    for dy in range(3)], axis=1)  # [128, 3, 64]
    w2s = np.ascontiguousarray(w2t[:, [2, 5, 8], :])  # [64, 3, 64] taps (dy,2)
    w3t = np.ascontiguousarray(w3[:, :, 0, 0].T).astype(np.float32)
    b3s = np.ascontiguousarray(b3.reshape(2, 128).T).astype(np.float32)
    b1d = np.concatenate([b1.reshape(MID, 1)] * 2, 0).astype(np.float32)
    return dict(w1t=w1d, w2p=w2p, w2s=w2s, w3t=w3t,
                b1=b1d,
                b2=b2.reshape(MID, 1).astype(np.float32),
                b3s=b3s, b3x4=(4.0 * b3s).astype(np.float32))


_nc_cache = None


def kernel(features, w1, b1, w2, b2, w3, b3, enabled):
    global _nc_cache, LAST_RESULTS
    features = np.asarray(features, np.float32)
    if not int(np.asarray(enabled)):
        return (np.ones((B, C, H, W), np.float32),
                np.zeros((B, C, H, W), np.float32))
    if _nc_cache is None:
        _nc_cache = build_nc()
    nc = _nc_cache
    wmap = _prep_weights(np.asarray(w1), np.asarray(b1), np.asarray(w2),
                         np.asarray(b2), np.asarray(w3), np.asarray(b3))
    fr = features.reshape(B, C, HW)
    in_maps = [dict(features=fr[c * IMGS:(c + 1) * IMGS], **wmap)
               for c in range(N_CORES)]
    res = bass_utils.run_bass_kernel_spmd(nc, in_maps, list(range(N_CORES)),
                                          trace=TRACE)
    LAST_RESULTS = res
    mask = np.concatenate([res.results[c]["mask"] for c in range(N_CORES)], 0)
    scores = np.concatenate([res.results[c]["scores"] for c in range(N_CORES)], 0)
    return (mask.reshape(B, C, H, W).astype(np.float32),
            scores.reshape(B, C, H, W).astype(np.float32))


if __name__ == "__main__":
    nc = build_nc()
    print("build + compile OK")

